# revision 3
# baseline (speedup 1.0000x reference)
"""IslandLoss layer as a Bass/Tile kernel on 8 Trainium2 NeuronCores.

Math (reference):
    Centers  = labels @ centers                 # (B,D) per-sample center
    delta    = labels.T @ (Centers - feats) / (counts+1)
    result_i = 0.5*||feats_i - Centers_i||^2 + pair_dist          # (B,1)
    new_centers = centers - ALPHA*delta + (L2/(C-1)) * T(centers)

Sharding: data-parallel over B across 8 cores (4096 rows each).
Per-core partial sums S = labels.T @ feats (7,1024) and counts (7,1) are
AllReduced; delta = (counts*centers - S)/(counts+1) uses the identity
labels.T @ labels @ centers = diag(counts) @ centers.
"""

import sys

sys.path.insert(0, "/opt/trn_rl_repo")

import numpy as np

import concourse.bacc as bacc
import concourse.bass as bass
import concourse.tile as tile
from concourse import mybir

B = 32768
D = 1024
C = 7
ALPHA = 0.5
LAMBDA2 = 0.5
NCORES = 8
SHARD = B // NCORES  # 4096

FP = mybir.dt.float32
Alu = mybir.AluOpType
Act = mybir.ActivationFunctionType


def build_island_nc(shard: int = SHARD, ncores: int = NCORES):
    """Build + compile the Bass module. Same program on every core."""
    assert shard % 128 == 0
    ntiles = shard // 128

    nc = bacc.Bacc(
        "TRN2",
        target_bir_lowering=False,
        debug=False,
        enable_asserts=False,
        num_devices=ncores,
    )

    feats = nc.dram_tensor("feats", [shard, D], FP, kind="ExternalInput").ap()
    labels = nc.dram_tensor("labels", [shard, C], FP, kind="ExternalInput").ap()
    centers = nc.dram_tensor("centers", [C, D], FP, kind="ExternalInput").ap()
    eye7 = nc.dram_tensor("eye7", [C, C], FP, kind="ExternalInput").ap()
    offmask = nc.dram_tensor("offmask", [C, C], FP, kind="ExternalInput").ap()
    result = nc.dram_tensor("result", [shard, 1], FP, kind="ExternalOutput").ap()
    new_centers = nc.dram_tensor(
        "new_centers", [C, D], FP, kind="ExternalOutput"
    ).ap()

    with tile.TileContext(nc) as tc:
        _island_body(tc, feats, labels, centers, eye7, offmask, result,
                     new_centers, ntiles, ncores)

    nc.compile()
    return nc


def _island_body(tc, feats, labels, centers, eye7, offmask, result,
                 new_centers, ntiles, ncores):
    nc = tc.nc
    KD = D // 128  # 8 chunks of centers.T

    import contextlib

    ctx = contextlib.ExitStack()
    with ctx:
        persist = ctx.enter_context(tc.tile_pool(name="persist", bufs=1))
        psum_acc = ctx.enter_context(tc.tile_pool(name="psum_acc", bufs=1, space="PSUM"))

        # ---------- persistent SBUF tiles ----------
        centers_sb = persist.tile([C, D], FP)
        nc.sync.dma_start(out=centers_sb, in_=centers)

        centersT_sb = persist.tile([128, KD, C], FP)  # centers.T in 8 chunks
        centersT_view = centers.transpose([1, 0])  # (D, C) strided view
        for k in range(KD):
            nc.sync.dma_start(
                out=centersT_sb[:, k, :],
                in_=centersT_view[k * 128:(k + 1) * 128, :],
            )

        eye7_sb = persist.tile([C, C], FP)
        nc.sync.dma_start(out=eye7_sb, in_=eye7)
        off_sb = persist.tile([C, C], FP)
        nc.sync.dma_start(out=off_sb, in_=offmask)

        labels_sb = persist.tile([128, ntiles, C], FP)
        lab_re = labels.rearrange("(t p) c -> p t c", p=128)
        nc.sync.dma_start(out=labels_sb, in_=lab_re)

        labelsT_sb = persist.tile([C, ntiles * 128], FP)
        labT_view = labels.transpose([1, 0])  # (C, shard) strided view
        nchunk = max(1, (ntiles * 128) // 512)
        csz = (ntiles * 128) // nchunk
        for j in range(nchunk):
            nc.sync.dma_start(
                out=labelsT_sb[:, j * csz:(j + 1) * csz],
                in_=labT_view[:, j * csz:(j + 1) * csz],
            )

        ones7 = persist.tile([C, 1], FP)
        nc.vector.memset(ones7, 1.0)
        ones7row = persist.tile([1, C], FP)
        nc.vector.memset(ones7row, 1.0)
        ones128row = persist.tile([1, 128], FP)
        nc.vector.memset(ones128row, 1.0)
        ones_col = persist.tile([128, 1], FP)
        nc.vector.memset(ones_col, 1.0)

        ssq_buf = persist.tile([128, ntiles], FP)
        pair_bias = persist.tile([128, 1], FP)
        base_sb = persist.tile([C, D], FP)  # centers + island term
        inv_sb = persist.tile([C, 1], FP)

        # ---------- prologue: centers-only math ----------
        with tc.tile_pool(name="pro_ps", bufs=1, space="PSUM") as pro_ps, \
             tc.tile_pool(name="pro_sb", bufs=2) as pro_sb:
            # G = centers @ centers.T  (7,7)
            g_ps = pro_ps.tile([C, C], FP, tag="tiny")
            for k in range(KD):
                nc.tensor.matmul(
                    g_ps, centersT_sb[:, k, :], centersT_sb[:, k, :],
                    start=(k == 0), stop=(k == KD - 1),
                )
            g_sb = pro_sb.tile([C, C], FP)
            nc.vector.tensor_copy(g_sb, g_ps)

            # ns2 = row ||c||^2 ; n = sqrt ; inv = 1/n
            sq_scr = pro_sb.tile([C, D], FP)
            ns2 = pro_sb.tile([C, 1], FP)
            nc.scalar.activation(sq_scr, centers_sb, Act.Square, accum_out=ns2)
            n_sb = pro_sb.tile([C, 1], FP)
            nc.scalar.sqrt(n_sb, ns2)
            nc.vector.reciprocal(inv_sb, n_sb)

            # invsum broadcast to (7,1)
            is_ps = pro_ps.tile([1, 1], FP, tag="tiny")
            nc.tensor.matmul(is_ps, inv_sb, ones7, start=True, stop=True)
            is_sb = pro_sb.tile([1, 1], FP)
            nc.scalar.copy(is_sb, is_ps)
            isb_ps = pro_ps.tile([C, 1], FP, tag="tiny")
            nc.tensor.matmul(isb_ps, ones7row, is_sb, start=True, stop=True)
            invsum_b = pro_sb.tile([C, 1], FP)
            nc.scalar.copy(invsum_b, isb_ps)

            # invrow (7,7): replicate inv^T on all partitions
            invT_ps = pro_ps.tile([1, C], FP, tag="tiny")
            nc.tensor.transpose(invT_ps, inv_sb, eye7_sb)
            invT_sb = pro_sb.tile([1, C], FP)
            nc.scalar.copy(invT_sb, invT_ps)
            invrow_ps = pro_ps.tile([C, C], FP, tag="tiny")
            nc.tensor.matmul(invrow_ps, ones7row, invT_sb, start=True, stop=True)
            invrow = pro_sb.tile([C, C], FP)
            nc.scalar.copy(invrow, invrow_ps)

            # pair_dist = L2 * sum((cos+1)*off), cos = G*invcol*invrow
            cos_sb = pro_sb.tile([C, C], FP)
            nc.vector.tensor_scalar_mul(cos_sb, g_sb, inv_sb)
            nc.vector.tensor_mul(cos_sb, cos_sb, invrow)
            pairm = pro_sb.tile([C, C], FP)
            nc.vector.scalar_tensor_tensor(
                pairm, cos_sb, 1.0, off_sb, op0=Alu.add, op1=Alu.mult
            )
            prs = pro_sb.tile([C, 1], FP)
            nc.vector.tensor_reduce(prs, pairm, mybir.AxisListType.X, Alu.add)
            pt_ps = pro_ps.tile([1, 1], FP, tag="tiny")
            nc.tensor.matmul(pt_ps, prs, ones7, start=True, stop=True)
            pt_sb = pro_sb.tile([1, 1], FP)
            nc.scalar.mul(pt_sb, pt_ps, LAMBDA2)
            pb_ps = pro_ps.tile([128, 1], FP, tag="tiny")
            nc.tensor.matmul(pb_ps, ones128row, pt_sb, start=True, stop=True)
            nc.scalar.copy(pair_bias, pb_ps)

            # coef1 = inv * (invsum - inv); inv3 = inv^3
            coef1 = pro_sb.tile([C, 1], FP)
            nc.vector.tensor_sub(coef1, invsum_b, inv_sb)
            nc.vector.tensor_mul(coef1, coef1, inv_sb)
            inv3 = pro_sb.tile([C, 1], FP)
            nc.vector.tensor_mul(inv3, inv_sb, inv_sb)
            nc.vector.tensor_mul(inv3, inv3, inv_sb)

            # coef2T[m,j] = G[m,j]*inv3[m]*inv[j]*off  (lhsT for T2)
            c2t = pro_sb.tile([C, C], FP)
            nc.vector.tensor_scalar_mul(c2t, g_sb, inv3)
            nc.vector.tensor_mul(c2t, c2t, invrow)
            nc.vector.tensor_mul(c2t, c2t, off_sb)

            # T2 = coef2 @ centers ; base = centers + (L2/(C-1))*(coef1*centers - T2)
            t2_ps = pro_ps.tile([C, D], FP)
            for j in range(D // 512):
                nc.tensor.matmul(
                    t2_ps[:, j * 512:(j + 1) * 512],
                    c2t, centers_sb[:, j * 512:(j + 1) * 512],
                    start=True, stop=True,
                )
            tt = pro_sb.tile([C, D], FP)
            nc.vector.scalar_tensor_tensor(
                tt, centers_sb, coef1, t2_ps, op0=Alu.mult, op1=Alu.subtract
            )
            nc.vector.scalar_tensor_tensor(
                base_sb, tt, LAMBDA2 / (C - 1), centers_sb,
                op0=Alu.mult, op1=Alu.add,
            )

        # ---------- main loop over 128-row tiles ----------
        s_ps = psum_acc.tile([C, D], FP)       # += labels.T @ feats
        cnt_ps = psum_acc.tile([C, 1], FP)     # += labels.T @ ones

        feats_re = feats.rearrange("(t p) d -> p t d", p=128)

        with tc.tile_pool(name="ld", bufs=3) as ld, \
             tc.tile_pool(name="work", bufs=2) as work, \
             tc.tile_pool(name="psum_w", bufs=2, space="PSUM") as psw:
            for t in range(ntiles):
                f_tile = ld.tile([128, D], FP)
                nc.sync.dma_start(out=f_tile, in_=feats_re[:, t, :])

                # per-sample centers: labels @ centers
                c_ps = psw.tile([128, D], FP)
                labT = labelsT_sb[:, t * 128:(t + 1) * 128]
                for j in range(D // 512):
                    nc.tensor.matmul(
                        c_ps[:, j * 512:(j + 1) * 512],
                        labT, centers_sb[:, j * 512:(j + 1) * 512],
                        start=True, stop=True,
                    )

                # diff and 0.5-less row ssq
                diff = work.tile([128, D], FP)
                nc.vector.tensor_sub(diff, f_tile, c_ps)
                sq = work.tile([128, D], FP)
                nc.scalar.activation(
                    sq, diff, Act.Square, accum_out=ssq_buf[:, t:t + 1]
                )

                # partial sums for the center update
                lab = labels_sb[:, t, :]
                for j in range(D // 512):
                    nc.tensor.matmul(
                        s_ps[:, j * 512:(j + 1) * 512],
                        lab, f_tile[:, j * 512:(j + 1) * 512],
                        start=(t == 0), stop=(t == ntiles - 1),
                    )
                nc.tensor.matmul(
                    cnt_ps, lab, ones_col,
                    start=(t == 0), stop=(t == ntiles - 1),
                )

        # result = 0.5*ssq + pair_dist
        res_sb = persist.tile([128, ntiles], FP)
        nc.scalar.activation(
            res_sb, ssq_buf, Act.Identity, bias=pair_bias, scale=0.5
        )
        res_view = result.rearrange("(t p) o -> p (t o)", p=128)
        nc.sync.dma_start(out=res_view, in_=res_sb)

        # ---------- AllReduce partial sums ----------
        CCW = D + 8  # 32B-aligned row
        cc_sb = persist.tile([C, CCW], FP)
        nc.vector.memset(cc_sb[:, D:CCW], 0.0)
        nc.vector.tensor_copy(cc_sb[:, 0:D], s_ps)
        nc.vector.tensor_copy(cc_sb[:, D:D + 1], cnt_ps)

        with tc.tile_pool(name="dram", bufs=1, space="DRAM") as dram:
            cc_in = dram.tile([C, CCW], FP)
            cc_out = dram.tile([C, CCW], FP)
            nc.gpsimd.dma_start(out=cc_in, in_=cc_sb)
            nc.gpsimd.collective_compute(
                "AllReduce",
                Alu.add,
                replica_groups=[list(range(ncores))],
                ins=[cc_in.opt()],
                outs=[cc_out.opt()],
            )
            red_sb = persist.tile([C, CCW], FP)
            nc.gpsimd.dma_start(out=red_sb, in_=cc_out)

        # ---------- center update ----------
        # delta = (counts*centers - S) / (counts + 1)
        numer = persist.tile([C, D], FP)
        nc.vector.scalar_tensor_tensor(
            numer, centers_sb, red_sb[:, D:D + 1], red_sb[:, 0:D],
            op0=Alu.mult, op1=Alu.subtract,
        )
        dcol = persist.tile([C, 1], FP)
        nc.vector.tensor_scalar_add(dcol, red_sb[:, D:D + 1], 1.0)
        nc.vector.reciprocal(dcol, dcol)
        nc.vector.tensor_scalar_mul(dcol, dcol, -ALPHA)
        ncent = persist.tile([C, D], FP)
        nc.vector.scalar_tensor_tensor(
            ncent, numer, dcol, base_sb, op0=Alu.mult, op1=Alu.add
        )
        nc.sync.dma_start(out=new_centers, in_=ncent)


_NC_CACHE = {}


def _get_nc(shard=SHARD, ncores=NCORES):
    key = (shard, ncores)
    if key not in _NC_CACHE:
        _NC_CACHE[key] = build_island_nc(shard, ncores)
    return _NC_CACHE[key]


def _run_spmd(feats, labels, centers, **spmd_kwargs):
    from concourse import bass_utils

    feats = np.ascontiguousarray(feats, dtype=np.float32)
    labels = np.ascontiguousarray(labels, dtype=np.float32)
    centers = np.ascontiguousarray(centers, dtype=np.float32)

    nc = _get_nc()
    eye = np.eye(C, dtype=np.float32)
    off = (1.0 - eye).astype(np.float32)

    in_maps = []
    for r in range(NCORES):
        in_maps.append({
            "feats": feats[r * SHARD:(r + 1) * SHARD],
            "labels": labels[r * SHARD:(r + 1) * SHARD],
            "centers": centers,
            "eye7": eye,
            "offmask": off,
        })

    res = bass_utils.run_bass_kernel_spmd(
        nc, in_maps, core_ids=list(range(NCORES)), **spmd_kwargs
    )
    outs = res.results
    result = np.concatenate([outs[r]["result"] for r in range(NCORES)], axis=0)
    new_centers = outs[0]["new_centers"]
    return res, (result, new_centers)


def kernel(feats: np.ndarray, labels: np.ndarray, centers: np.ndarray):
    _, out = _run_spmd(feats, labels, centers)
    return out


if __name__ == "__main__":
    rng = np.random.default_rng(0)
    f = rng.standard_normal((B, D), dtype=np.float32)
    lbl = rng.integers(0, C, size=B)
    lab = np.eye(C, dtype=np.float32)[lbl]
    cen = rng.uniform(-0.05, 0.05, size=(C, D)).astype(np.float32)
    r, ncent = kernel(f, lab, cen)
    print(r.shape, ncent.shape, r[:4, 0])


# revision 8
# speedup vs baseline: 1.4125x; 1.4125x over previous
"""IslandLoss layer as a Bass/Tile kernel on 8 Trainium2 NeuronCores.

Math (reference):
    Centers  = labels @ centers                 # (B,D) per-sample center
    delta    = labels.T @ (Centers - feats) / (counts+1)
    result_i = 0.5*||feats_i - Centers_i||^2 + pair_dist          # (B,1)
    new_centers = centers - ALPHA*delta + (L2/(C-1)) * T(centers)

Sharding: data-parallel over B across 8 cores (4096 rows each).
Per-core partial sums S = labels.T @ feats (7,1024) and counts (7,1) are
AllReduced; delta = (counts*centers - S)/(counts+1) uses the identity
labels.T @ labels @ centers = diag(counts) @ centers.
"""

import sys

sys.path.insert(0, "/opt/trn_rl_repo")

import numpy as np

import concourse.bacc as bacc
import concourse.bass as bass
import concourse.tile as tile
from concourse import mybir

B = 32768
D = 1024
C = 7
ALPHA = 0.5
LAMBDA2 = 0.5
NCORES = 8
SHARD = B // NCORES  # 4096

FP = mybir.dt.float32
BF = mybir.dt.bfloat16
Alu = mybir.AluOpType
Act = mybir.ActivationFunctionType


def build_island_nc(shard: int = SHARD, ncores: int = NCORES):
    """Build + compile the Bass module. Same program on every core."""
    assert shard % 128 == 0
    ntiles = shard // 128

    nc = bacc.Bacc(
        "TRN2",
        target_bir_lowering=False,
        debug=False,
        enable_asserts=False,
        num_devices=ncores,
    )

    feats = nc.dram_tensor("feats", [shard, D], FP, kind="ExternalInput").ap()
    labels = nc.dram_tensor("labels", [shard, C], FP, kind="ExternalInput").ap()
    centers = nc.dram_tensor("centers", [C, D], FP, kind="ExternalInput").ap()
    eye7 = nc.dram_tensor("eye7", [C, C], FP, kind="ExternalInput").ap()
    offmask = nc.dram_tensor("offmask", [C, C], FP, kind="ExternalInput").ap()
    eye128 = nc.dram_tensor("eye128", [128, 128], FP, kind="ExternalInput").ap()
    result = nc.dram_tensor("result", [shard, 1], FP, kind="ExternalOutput").ap()
    new_centers = nc.dram_tensor(
        "new_centers", [C, D], FP, kind="ExternalOutput"
    ).ap()

    with tile.TileContext(nc) as tc:
        _island_body(tc, feats, labels, centers, eye7, offmask, eye128,
                     result, new_centers, ntiles, ncores)

    nc.compile()
    return nc


def _island_body(tc, feats, labels, centers, eye7, offmask, eye128,
                 result, new_centers, ntiles, ncores):
    nc = tc.nc
    KD = D // 128  # 8 chunks of centers.T

    import contextlib

    ctx = contextlib.ExitStack()
    with ctx:
        persist = ctx.enter_context(tc.tile_pool(name="persist", bufs=1))
        psum_acc = ctx.enter_context(tc.tile_pool(name="psum_acc", bufs=1, space="PSUM"))

        # ---------- persistent SBUF tiles ----------
        centers_sb = persist.tile([C, D], FP)
        nc.sync.dma_start(out=centers_sb, in_=centers)

        centersT_sb = persist.tile([128, KD, C], FP)  # centers.T in 8 chunks
        centersT_view = centers.transpose([1, 0])  # (D, C) strided view
        for k in range(KD):
            nc.sync.dma_start(
                out=centersT_sb[:, k, :],
                in_=centersT_view[k * 128:(k + 1) * 128, :],
            )

        eye7_sb = persist.tile([C, C], FP)
        nc.sync.dma_start(out=eye7_sb, in_=eye7)
        off_sb = persist.tile([C, C], FP)
        nc.sync.dma_start(out=off_sb, in_=offmask)

        # interleaved row mapping: partition p of tile t holds row p*ntiles+t,
        # so every DMA below is contiguous per partition.
        labels_sb = persist.tile([128, ntiles, C], FP)
        lab_re = labels.rearrange("(p t) c -> p t c", p=128)
        nc.sync.dma_start(out=labels_sb, in_=lab_re)
        labels_bf = persist.tile([128, ntiles, C], BF)
        nc.vector.tensor_copy(labels_bf, labels_sb)

        centers_bf = persist.tile([C, D], BF)
        nc.vector.tensor_copy(centers_bf, centers_sb)

        eye128_sb = persist.tile([128, 128], FP)
        nc.sync.dma_start(out=eye128_sb, in_=eye128)
        eye128_bf = persist.tile([128, 128], BF)
        nc.vector.tensor_copy(eye128_bf, eye128_sb)

        ones7 = persist.tile([C, 1], FP)
        nc.vector.memset(ones7, 1.0)
        ones7row = persist.tile([1, C], FP)
        nc.vector.memset(ones7row, 1.0)
        ones128row = persist.tile([1, 128], FP)
        nc.vector.memset(ones128row, 1.0)
        ones_col_bf = persist.tile([128, 1], BF)
        nc.vector.memset(ones_col_bf, 1.0)

        ssq_buf = persist.tile([128, ntiles], FP)
        pair_bias = persist.tile([128, 1], FP)
        base_sb = persist.tile([C, D], FP)  # centers + island term
        inv_sb = persist.tile([C, 1], FP)

        # ---------- prologue: centers-only math ----------
        with tc.tile_pool(name="pro_ps", bufs=1, space="PSUM") as pro_ps, \
             tc.tile_pool(name="pro_sb", bufs=2) as pro_sb:
            # G = centers @ centers.T  (7,7)
            g_ps = pro_ps.tile([C, C], FP, tag="tiny")
            for k in range(KD):
                nc.tensor.matmul(
                    g_ps, centersT_sb[:, k, :], centersT_sb[:, k, :],
                    start=(k == 0), stop=(k == KD - 1),
                )
            g_sb = pro_sb.tile([C, C], FP)
            nc.vector.tensor_copy(g_sb, g_ps)

            # ns2 = row ||c||^2 ; n = sqrt ; inv = 1/n
            sq_scr = pro_sb.tile([C, D], FP)
            ns2 = pro_sb.tile([C, 1], FP)
            nc.scalar.activation(sq_scr, centers_sb, Act.Square, accum_out=ns2)
            n_sb = pro_sb.tile([C, 1], FP)
            nc.scalar.sqrt(n_sb, ns2)
            nc.vector.reciprocal(inv_sb, n_sb)

            # invsum broadcast to (7,1)
            is_ps = pro_ps.tile([1, 1], FP, tag="tiny")
            nc.tensor.matmul(is_ps, inv_sb, ones7, start=True, stop=True)
            is_sb = pro_sb.tile([1, 1], FP)
            nc.scalar.copy(is_sb, is_ps)
            isb_ps = pro_ps.tile([C, 1], FP, tag="tiny")
            nc.tensor.matmul(isb_ps, ones7row, is_sb, start=True, stop=True)
            invsum_b = pro_sb.tile([C, 1], FP)
            nc.scalar.copy(invsum_b, isb_ps)

            # invrow (7,7): replicate inv^T on all partitions
            invT_ps = pro_ps.tile([1, C], FP, tag="tiny")
            nc.tensor.transpose(invT_ps, inv_sb, eye7_sb)
            invT_sb = pro_sb.tile([1, C], FP)
            nc.scalar.copy(invT_sb, invT_ps)
            invrow_ps = pro_ps.tile([C, C], FP, tag="tiny")
            nc.tensor.matmul(invrow_ps, ones7row, invT_sb, start=True, stop=True)
            invrow = pro_sb.tile([C, C], FP)
            nc.scalar.copy(invrow, invrow_ps)

            # pair_dist = L2 * sum((cos+1)*off), cos = G*invcol*invrow
            cos_sb = pro_sb.tile([C, C], FP)
            nc.vector.tensor_scalar_mul(cos_sb, g_sb, inv_sb)
            nc.vector.tensor_mul(cos_sb, cos_sb, invrow)
            pairm = pro_sb.tile([C, C], FP)
            nc.vector.scalar_tensor_tensor(
                pairm, cos_sb, 1.0, off_sb, op0=Alu.add, op1=Alu.mult
            )
            prs = pro_sb.tile([C, 1], FP)
            nc.vector.tensor_reduce(prs, pairm, mybir.AxisListType.X, Alu.add)
            pt_ps = pro_ps.tile([1, 1], FP, tag="tiny")
            nc.tensor.matmul(pt_ps, prs, ones7, start=True, stop=True)
            pt_sb = pro_sb.tile([1, 1], FP)
            nc.scalar.mul(pt_sb, pt_ps, LAMBDA2)
            pb_ps = pro_ps.tile([128, 1], FP, tag="tiny")
            nc.tensor.matmul(pb_ps, ones128row, pt_sb, start=True, stop=True)
            nc.scalar.copy(pair_bias, pb_ps)

            # coef1 = inv * (invsum - inv); inv3 = inv^3
            coef1 = pro_sb.tile([C, 1], FP)
            nc.vector.tensor_sub(coef1, invsum_b, inv_sb)
            nc.vector.tensor_mul(coef1, coef1, inv_sb)
            inv3 = pro_sb.tile([C, 1], FP)
            nc.vector.tensor_mul(inv3, inv_sb, inv_sb)
            nc.vector.tensor_mul(inv3, inv3, inv_sb)

            # coef2T[m,j] = G[m,j]*inv3[m]*inv[j]*off  (lhsT for T2)
            c2t = pro_sb.tile([C, C], FP)
            nc.vector.tensor_scalar_mul(c2t, g_sb, inv3)
            nc.vector.tensor_mul(c2t, c2t, invrow)
            nc.vector.tensor_mul(c2t, c2t, off_sb)

            # T2 = coef2 @ centers ; base = centers + (L2/(C-1))*(coef1*centers - T2)
            t2_ps = pro_ps.tile([C, D], FP)
            for j in range(D // 512):
                nc.tensor.matmul(
                    t2_ps[:, j * 512:(j + 1) * 512],
                    c2t, centers_sb[:, j * 512:(j + 1) * 512],
                    start=True, stop=True,
                )
            tt = pro_sb.tile([C, D], FP)
            nc.vector.scalar_tensor_tensor(
                tt, centers_sb, coef1, t2_ps, op0=Alu.mult, op1=Alu.subtract
            )
            nc.vector.scalar_tensor_tensor(
                base_sb, tt, LAMBDA2 / (C - 1), centers_sb,
                op0=Alu.mult, op1=Alu.add,
            )

        # ---------- main loop over 128-row tiles ----------
        s_ps = psum_acc.tile([C, D], FP)       # += labels.T @ feats
        cnt_ps = psum_acc.tile([C, 1], FP)     # += labels.T @ ones

        feats_re = feats.rearrange("(p t) d -> p t d", p=128)

        with tc.tile_pool(name="ld", bufs=3) as ld, \
             tc.tile_pool(name="work", bufs=2) as work, \
             tc.tile_pool(name="psum_w", bufs=2, space="PSUM") as psw, \
             tc.tile_pool(name="psum_t", bufs=1, space="PSUM") as pst:
            for t in range(ntiles):
                f_tile = ld.tile([128, D], FP)
                nc.sync.dma_start(out=f_tile, in_=feats_re[:, t, :])

                # labels tile transposed on-chip (PE) -> bf16 lhsT for mm1
                labT_ps = pst.tile([C, 128], BF, tag="labT")
                nc.tensor.transpose(labT_ps, labels_bf[:, t, :], eye128_bf)
                labT_sb = work.tile([C, 128], BF)
                nc.vector.tensor_copy(labT_sb, labT_ps)

                # per-sample centers: labels @ centers (bf16 in, f32 acc)
                c_ps = psw.tile([128, D], FP)
                for j in range(D // 512):
                    nc.tensor.matmul(
                        c_ps[:, j * 512:(j + 1) * 512],
                        labT_sb, centers_bf[:, j * 512:(j + 1) * 512],
                        start=True, stop=True,
                    )

                # diff and 0.5-less row ssq
                diff = work.tile([128, D], FP)
                nc.vector.tensor_sub(diff, f_tile, c_ps)
                sq = work.tile([128, D], FP)
                nc.scalar.activation(
                    sq, diff, Act.Square, accum_out=ssq_buf[:, t:t + 1]
                )

                # S += labels.T @ feats in bf16 (one-hot weights exact);
                # gpsimd (otherwise idle) makes the bf16 copy of feats
                f_bf = work.tile([128, D], BF)
                nc.gpsimd.tensor_copy(f_bf, f_tile)
                lab = labels_bf[:, t, :]
                for j in range(D // 512):
                    nc.tensor.matmul(
                        s_ps[:, j * 512:(j + 1) * 512],
                        lab, f_bf[:, j * 512:(j + 1) * 512],
                        start=(t == 0), stop=(t == ntiles - 1),
                    )
                nc.tensor.matmul(
                    cnt_ps, lab, ones_col_bf,
                    start=(t == 0), stop=(t == ntiles - 1),
                )

        # result = 0.5*ssq + pair_dist
        res_sb = persist.tile([128, ntiles], FP)
        nc.scalar.activation(
            res_sb, ssq_buf, Act.Identity, bias=pair_bias, scale=0.5
        )
        res_view = result.rearrange("(p t) o -> p (t o)", p=128)
        nc.sync.dma_start(out=res_view, in_=res_sb)

        # ---------- AllReduce partial sums ----------
        CCW = D + 8  # 32B-aligned row
        cc_sb = persist.tile([C, CCW], FP)
        nc.vector.memset(cc_sb[:, D:CCW], 0.0)
        nc.vector.tensor_copy(cc_sb[:, 0:D], s_ps)
        nc.vector.tensor_copy(cc_sb[:, D:D + 1], cnt_ps)

        with tc.tile_pool(name="dram", bufs=1, space="DRAM") as dram:
            cc_in = dram.tile([C, CCW], FP)
            cc_out = dram.tile([C, CCW], FP)
            nc.gpsimd.dma_start(out=cc_in, in_=cc_sb)
            nc.gpsimd.collective_compute(
                "AllReduce",
                Alu.add,
                replica_groups=[list(range(ncores))],
                ins=[cc_in.opt()],
                outs=[cc_out.opt()],
            )
            red_sb = persist.tile([C, CCW], FP)
            nc.gpsimd.dma_start(out=red_sb, in_=cc_out)

        # ---------- center update ----------
        # delta = (counts*centers - S) / (counts + 1)
        numer = persist.tile([C, D], FP)
        nc.vector.scalar_tensor_tensor(
            numer, centers_sb, red_sb[:, D:D + 1], red_sb[:, 0:D],
            op0=Alu.mult, op1=Alu.subtract,
        )
        dcol = persist.tile([C, 1], FP)
        nc.vector.tensor_scalar_add(dcol, red_sb[:, D:D + 1], 1.0)
        nc.vector.reciprocal(dcol, dcol)
        nc.vector.tensor_scalar_mul(dcol, dcol, -ALPHA)
        ncent = persist.tile([C, D], FP)
        nc.vector.scalar_tensor_tensor(
            ncent, numer, dcol, base_sb, op0=Alu.mult, op1=Alu.add
        )
        nc.sync.dma_start(out=new_centers, in_=ncent)


_NC_CACHE = {}


def _get_nc(shard=SHARD, ncores=NCORES):
    key = (shard, ncores)
    if key not in _NC_CACHE:
        _NC_CACHE[key] = build_island_nc(shard, ncores)
    return _NC_CACHE[key]


def _run_spmd(feats, labels, centers, **spmd_kwargs):
    from concourse import bass_utils

    feats = np.ascontiguousarray(feats, dtype=np.float32)
    labels = np.ascontiguousarray(labels, dtype=np.float32)
    centers = np.ascontiguousarray(centers, dtype=np.float32)

    nc = _get_nc()
    eye = np.eye(C, dtype=np.float32)
    off = (1.0 - eye).astype(np.float32)

    in_maps = []
    for r in range(NCORES):
        in_maps.append({
            "feats": feats[r * SHARD:(r + 1) * SHARD],
            "labels": labels[r * SHARD:(r + 1) * SHARD],
            "centers": centers,
            "eye7": eye,
            "offmask": off,
            "eye128": np.eye(128, dtype=np.float32),
        })

    res = bass_utils.run_bass_kernel_spmd(
        nc, in_maps, core_ids=list(range(NCORES)), **spmd_kwargs
    )
    outs = res.results
    result = np.concatenate([outs[r]["result"] for r in range(NCORES)], axis=0)
    new_centers = outs[0]["new_centers"]
    return res, (result, new_centers)


def kernel(feats: np.ndarray, labels: np.ndarray, centers: np.ndarray):
    _, out = _run_spmd(feats, labels, centers)
    return out


if __name__ == "__main__":
    rng = np.random.default_rng(0)
    f = rng.standard_normal((B, D), dtype=np.float32)
    lbl = rng.integers(0, C, size=B)
    lab = np.eye(C, dtype=np.float32)[lbl]
    cen = rng.uniform(-0.05, 0.05, size=(C, D)).astype(np.float32)
    r, ncent = kernel(f, lab, cen)
    print(r.shape, ncent.shape, r[:4, 0])


# revision 9
# speedup vs baseline: 1.8007x; 1.2748x over previous
"""IslandLoss layer as a Bass/Tile kernel on 8 Trainium2 NeuronCores.

Math (reference):
    Centers  = labels @ centers                 # (B,D) per-sample center
    delta    = labels.T @ (Centers - feats) / (counts+1)
    result_i = 0.5*||feats_i - Centers_i||^2 + pair_dist          # (B,1)
    new_centers = centers - ALPHA*delta + (L2/(C-1)) * T(centers)

Sharding: data-parallel over B across 8 cores (4096 rows each).
Per-core partial sums S = labels.T @ feats (7,1024) and counts (7,1) are
AllReduced; delta = (counts*centers - S)/(counts+1) uses the identity
labels.T @ labels @ centers = diag(counts) @ centers.
"""

import sys

sys.path.insert(0, "/opt/trn_rl_repo")

import numpy as np

import concourse.bacc as bacc
import concourse.bass as bass
import concourse.tile as tile
from concourse import mybir

B = 32768
D = 1024
C = 7
ALPHA = 0.5
LAMBDA2 = 0.5
NCORES = 8
SHARD = B // NCORES  # 4096

FP = mybir.dt.float32
BF = mybir.dt.bfloat16
Alu = mybir.AluOpType
Act = mybir.ActivationFunctionType


def build_island_nc(shard: int = SHARD, ncores: int = NCORES):
    """Build + compile the Bass module. Same program on every core."""
    assert shard % 128 == 0
    ntiles = shard // 128

    nc = bacc.Bacc(
        "TRN2",
        target_bir_lowering=False,
        debug=False,
        enable_asserts=False,
        num_devices=ncores,
    )

    feats = nc.dram_tensor("feats", [shard, D], FP, kind="ExternalInput").ap()
    labels = nc.dram_tensor("labels", [shard, C], FP, kind="ExternalInput").ap()
    centers = nc.dram_tensor("centers", [C, D], FP, kind="ExternalInput").ap()
    eye7 = nc.dram_tensor("eye7", [C, C], FP, kind="ExternalInput").ap()
    offmask = nc.dram_tensor("offmask", [C, C], FP, kind="ExternalInput").ap()
    eye128 = nc.dram_tensor("eye128", [128, 128], FP, kind="ExternalInput").ap()
    result = nc.dram_tensor("result", [shard, 1], FP, kind="ExternalOutput").ap()
    new_centers = nc.dram_tensor(
        "new_centers", [C, D], FP, kind="ExternalOutput"
    ).ap()

    with tile.TileContext(nc) as tc:
        _island_body(tc, feats, labels, centers, eye7, offmask, eye128,
                     result, new_centers, ntiles, ncores)

    nc.compile()
    return nc


def _island_body(tc, feats, labels, centers, eye7, offmask, eye128,
                 result, new_centers, ntiles, ncores):
    nc = tc.nc
    KD = D // 128  # 8 chunks of centers.T

    import contextlib

    ctx = contextlib.ExitStack()
    with ctx:
        persist = ctx.enter_context(tc.tile_pool(name="persist", bufs=1))
        psum_acc = ctx.enter_context(tc.tile_pool(name="psum_acc", bufs=1, space="PSUM"))

        # ---------- persistent SBUF tiles ----------
        centers_sb = persist.tile([C, D], FP)
        nc.sync.dma_start(out=centers_sb, in_=centers)

        centersT_sb = persist.tile([128, KD, C], FP)  # centers.T in 8 chunks
        centersT_view = centers.transpose([1, 0])  # (D, C) strided view
        for k in range(KD):
            nc.sync.dma_start(
                out=centersT_sb[:, k, :],
                in_=centersT_view[k * 128:(k + 1) * 128, :],
            )

        eye7_sb = persist.tile([C, C], FP)
        nc.sync.dma_start(out=eye7_sb, in_=eye7)
        off_sb = persist.tile([C, C], FP)
        nc.sync.dma_start(out=off_sb, in_=offmask)

        # interleaved row mapping: partition p of tile t holds row p*ntiles+t,
        # so every DMA below is contiguous per partition.
        labels_sb = persist.tile([128, ntiles, C], FP)
        lab_re = labels.rearrange("(p t) c -> p t c", p=128)
        nc.sync.dma_start(out=labels_sb, in_=lab_re)
        labels_bf = persist.tile([128, ntiles, C], BF)
        nc.vector.tensor_copy(labels_bf, labels_sb)

        centers_bf = persist.tile([C, D], BF)
        nc.vector.tensor_copy(centers_bf, centers_sb)

        eye128_sb = persist.tile([128, 128], FP)
        nc.sync.dma_start(out=eye128_sb, in_=eye128)
        eye128_bf = persist.tile([128, 128], BF)
        nc.vector.tensor_copy(eye128_bf, eye128_sb)

        ones7 = persist.tile([C, 1], FP)
        nc.vector.memset(ones7, 1.0)
        ones7row = persist.tile([1, C], FP)
        nc.vector.memset(ones7row, 1.0)
        ones128row = persist.tile([1, 128], FP)
        nc.vector.memset(ones128row, 1.0)
        ones_col = persist.tile([128, 1], FP)
        nc.vector.memset(ones_col, 1.0)

        ssq_buf = persist.tile([128, ntiles], FP)
        pair_bias = persist.tile([128, 1], FP)
        base_sb = persist.tile([C, D], FP)  # centers + island term
        inv_sb = persist.tile([C, 1], FP)

        # ---------- prologue: centers-only math ----------
        with tc.tile_pool(name="pro_ps", bufs=1, space="PSUM") as pro_ps, \
             tc.tile_pool(name="pro_sb", bufs=2) as pro_sb:
            # G = centers @ centers.T  (7,7)
            g_ps = pro_ps.tile([C, C], FP, tag="tiny")
            for k in range(KD):
                nc.tensor.matmul(
                    g_ps, centersT_sb[:, k, :], centersT_sb[:, k, :],
                    start=(k == 0), stop=(k == KD - 1),
                )
            g_sb = pro_sb.tile([C, C], FP)
            nc.vector.tensor_copy(g_sb, g_ps)

            # ns2 = row ||c||^2 ; n = sqrt ; inv = 1/n
            sq_scr = pro_sb.tile([C, D], FP)
            ns2 = pro_sb.tile([C, 1], FP)
            nc.scalar.activation(sq_scr, centers_sb, Act.Square, accum_out=ns2)
            n_sb = pro_sb.tile([C, 1], FP)
            nc.scalar.sqrt(n_sb, ns2)
            nc.vector.reciprocal(inv_sb, n_sb)

            # invsum broadcast to (7,1)
            is_ps = pro_ps.tile([1, 1], FP, tag="tiny")
            nc.tensor.matmul(is_ps, inv_sb, ones7, start=True, stop=True)
            is_sb = pro_sb.tile([1, 1], FP)
            nc.scalar.copy(is_sb, is_ps)
            isb_ps = pro_ps.tile([C, 1], FP, tag="tiny")
            nc.tensor.matmul(isb_ps, ones7row, is_sb, start=True, stop=True)
            invsum_b = pro_sb.tile([C, 1], FP)
            nc.scalar.copy(invsum_b, isb_ps)

            # invrow (7,7): replicate inv^T on all partitions
            invT_ps = pro_ps.tile([1, C], FP, tag="tiny")
            nc.tensor.transpose(invT_ps, inv_sb, eye7_sb)
            invT_sb = pro_sb.tile([1, C], FP)
            nc.scalar.copy(invT_sb, invT_ps)
            invrow_ps = pro_ps.tile([C, C], FP, tag="tiny")
            nc.tensor.matmul(invrow_ps, ones7row, invT_sb, start=True, stop=True)
            invrow = pro_sb.tile([C, C], FP)
            nc.scalar.copy(invrow, invrow_ps)

            # pair_dist = L2 * sum((cos+1)*off), cos = G*invcol*invrow
            cos_sb = pro_sb.tile([C, C], FP)
            nc.vector.tensor_scalar_mul(cos_sb, g_sb, inv_sb)
            nc.vector.tensor_mul(cos_sb, cos_sb, invrow)
            pairm = pro_sb.tile([C, C], FP)
            nc.vector.scalar_tensor_tensor(
                pairm, cos_sb, 1.0, off_sb, op0=Alu.add, op1=Alu.mult
            )
            prs = pro_sb.tile([C, 1], FP)
            nc.vector.tensor_reduce(prs, pairm, mybir.AxisListType.X, Alu.add)
            pt_ps = pro_ps.tile([1, 1], FP, tag="tiny")
            nc.tensor.matmul(pt_ps, prs, ones7, start=True, stop=True)
            pt_sb = pro_sb.tile([1, 1], FP)
            nc.scalar.mul(pt_sb, pt_ps, LAMBDA2)
            pb_ps = pro_ps.tile([128, 1], FP, tag="tiny")
            nc.tensor.matmul(pb_ps, ones128row, pt_sb, start=True, stop=True)
            nc.scalar.copy(pair_bias, pb_ps)

            # coef1 = inv * (invsum - inv); inv3 = inv^3
            coef1 = pro_sb.tile([C, 1], FP)
            nc.vector.tensor_sub(coef1, invsum_b, inv_sb)
            nc.vector.tensor_mul(coef1, coef1, inv_sb)
            inv3 = pro_sb.tile([C, 1], FP)
            nc.vector.tensor_mul(inv3, inv_sb, inv_sb)
            nc.vector.tensor_mul(inv3, inv3, inv_sb)

            # coef2T[m,j] = G[m,j]*inv3[m]*inv[j]*off  (lhsT for T2)
            c2t = pro_sb.tile([C, C], FP)
            nc.vector.tensor_scalar_mul(c2t, g_sb, inv3)
            nc.vector.tensor_mul(c2t, c2t, invrow)
            nc.vector.tensor_mul(c2t, c2t, off_sb)

            # T2 = coef2 @ centers ; base = centers + (L2/(C-1))*(coef1*centers - T2)
            t2_ps = pro_ps.tile([C, D], FP)
            for j in range(D // 512):
                nc.tensor.matmul(
                    t2_ps[:, j * 512:(j + 1) * 512],
                    c2t, centers_sb[:, j * 512:(j + 1) * 512],
                    start=True, stop=True,
                )
            tt = pro_sb.tile([C, D], FP)
            nc.vector.scalar_tensor_tensor(
                tt, centers_sb, coef1, t2_ps, op0=Alu.mult, op1=Alu.subtract
            )
            nc.vector.scalar_tensor_tensor(
                base_sb, tt, LAMBDA2 / (C - 1), centers_sb,
                op0=Alu.mult, op1=Alu.add,
            )

        # ---------- main loop over 128-row tiles ----------
        s_ps = psum_acc.tile([C, D], FP)       # += labels.T @ feats
        cnt_ps = psum_acc.tile([C, 1], FP)     # += labels.T @ ones

        feats_re = feats.rearrange("(p t) d -> p t d", p=128)

        with tc.tile_pool(name="ld", bufs=3) as ld, \
             tc.tile_pool(name="work", bufs=2) as work, \
             tc.tile_pool(name="psum_w", bufs=2, space="PSUM") as psw, \
             tc.tile_pool(name="psum_t", bufs=1, space="PSUM") as pst:
            for t in range(ntiles):
                f_tile = ld.tile([128, D], FP)
                nc.sync.dma_start(out=f_tile, in_=feats_re[:, t, :])

                # labels tile transposed on-chip (PE) -> bf16 lhsT for mm1
                labT_ps = pst.tile([C, 128], BF, tag="labT")
                nc.tensor.transpose(labT_ps, labels_bf[:, t, :], eye128_bf)
                labT_sb = work.tile([C, 128], BF)
                nc.vector.tensor_copy(labT_sb, labT_ps)

                # per-sample centers: labels @ centers (bf16 in, f32 acc)
                c_ps = psw.tile([128, D], FP)
                for j in range(D // 512):
                    nc.tensor.matmul(
                        c_ps[:, j * 512:(j + 1) * 512],
                        labT_sb, centers_bf[:, j * 512:(j + 1) * 512],
                        start=True, stop=True,
                    )

                # diff and 0.5-less row ssq
                diff = work.tile([128, D], FP)
                nc.vector.tensor_sub(diff, f_tile, c_ps)
                sq = work.tile([128, D], FP)
                nc.scalar.activation(
                    sq, diff, Act.Square, accum_out=ssq_buf[:, t:t + 1]
                )

                # S += labels.T @ feats in fp32 (full feats precision);
                # dual-pass fp32 matmul is affordable with mm1 in bf16
                lab = labels_sb[:, t, :]
                for j in range(D // 512):
                    nc.tensor.matmul(
                        s_ps[:, j * 512:(j + 1) * 512],
                        lab, f_tile[:, j * 512:(j + 1) * 512],
                        start=(t == 0), stop=(t == ntiles - 1),
                    )
                nc.tensor.matmul(
                    cnt_ps, lab, ones_col,
                    start=(t == 0), stop=(t == ntiles - 1),
                )

        # result = 0.5*ssq + pair_dist
        res_sb = persist.tile([128, ntiles], FP)
        nc.scalar.activation(
            res_sb, ssq_buf, Act.Identity, bias=pair_bias, scale=0.5
        )
        res_view = result.rearrange("(p t) o -> p (t o)", p=128)
        nc.sync.dma_start(out=res_view, in_=res_sb)

        # ---------- AllReduce partial sums ----------
        CCW = D + 8  # 32B-aligned row
        cc_sb = persist.tile([C, CCW], FP)
        nc.vector.memset(cc_sb[:, D:CCW], 0.0)
        nc.vector.tensor_copy(cc_sb[:, 0:D], s_ps)
        nc.vector.tensor_copy(cc_sb[:, D:D + 1], cnt_ps)

        with tc.tile_pool(name="dram", bufs=1, space="DRAM") as dram:
            cc_in = dram.tile([C, CCW], FP)
            cc_out = dram.tile([C, CCW], FP)
            nc.gpsimd.dma_start(out=cc_in, in_=cc_sb)
            nc.gpsimd.collective_compute(
                "AllReduce",
                Alu.add,
                replica_groups=[list(range(ncores))],
                ins=[cc_in.opt()],
                outs=[cc_out.opt()],
            )
            red_sb = persist.tile([C, CCW], FP)
            nc.gpsimd.dma_start(out=red_sb, in_=cc_out)

        # ---------- center update ----------
        # delta = (counts*centers - S) / (counts + 1)
        numer = persist.tile([C, D], FP)
        nc.vector.scalar_tensor_tensor(
            numer, centers_sb, red_sb[:, D:D + 1], red_sb[:, 0:D],
            op0=Alu.mult, op1=Alu.subtract,
        )
        dcol = persist.tile([C, 1], FP)
        nc.vector.tensor_scalar_add(dcol, red_sb[:, D:D + 1], 1.0)
        nc.vector.reciprocal(dcol, dcol)
        nc.vector.tensor_scalar_mul(dcol, dcol, -ALPHA)
        ncent = persist.tile([C, D], FP)
        nc.vector.scalar_tensor_tensor(
            ncent, numer, dcol, base_sb, op0=Alu.mult, op1=Alu.add
        )
        nc.sync.dma_start(out=new_centers, in_=ncent)


_NC_CACHE = {}


def _get_nc(shard=SHARD, ncores=NCORES):
    key = (shard, ncores)
    if key not in _NC_CACHE:
        _NC_CACHE[key] = build_island_nc(shard, ncores)
    return _NC_CACHE[key]


def _run_spmd(feats, labels, centers, **spmd_kwargs):
    from concourse import bass_utils

    feats = np.ascontiguousarray(feats, dtype=np.float32)
    labels = np.ascontiguousarray(labels, dtype=np.float32)
    centers = np.ascontiguousarray(centers, dtype=np.float32)

    nc = _get_nc()
    eye = np.eye(C, dtype=np.float32)
    off = (1.0 - eye).astype(np.float32)

    in_maps = []
    for r in range(NCORES):
        in_maps.append({
            "feats": feats[r * SHARD:(r + 1) * SHARD],
            "labels": labels[r * SHARD:(r + 1) * SHARD],
            "centers": centers,
            "eye7": eye,
            "offmask": off,
            "eye128": np.eye(128, dtype=np.float32),
        })

    res = bass_utils.run_bass_kernel_spmd(
        nc, in_maps, core_ids=list(range(NCORES)), **spmd_kwargs
    )
    outs = res.results
    result = np.concatenate([outs[r]["result"] for r in range(NCORES)], axis=0)
    new_centers = outs[0]["new_centers"]
    return res, (result, new_centers)


def kernel(feats: np.ndarray, labels: np.ndarray, centers: np.ndarray):
    _, out = _run_spmd(feats, labels, centers)
    return out


if __name__ == "__main__":
    rng = np.random.default_rng(0)
    f = rng.standard_normal((B, D), dtype=np.float32)
    lbl = rng.integers(0, C, size=B)
    lab = np.eye(C, dtype=np.float32)[lbl]
    cen = rng.uniform(-0.05, 0.05, size=(C, D)).astype(np.float32)
    r, ncent = kernel(f, lab, cen)
    print(r.shape, ncent.shape, r[:4, 0])
